# revision 24
# baseline (speedup 1.0000x reference)
"""Trainium2 Bass kernel for patch attention:
    out = softmax(silu(q) @ silu(k)^T * scale, axis=-1)
with q,k: [B=4, H=16, P=1024, D=128] fp32, scale: [1] fp32.

Sharding: B*H = 64 heads split across 8 NeuronCores, 8 heads each.

DMA-bound problem: per core 8.4 MB in + 33.5 MB out = 41.9 MB over
~358 GB/s -> ~117 us floor. The kernel is organized so the output-DMA
queue (sync) never stalls:

  - Inputs are DMA'd on the TENSOR engine queue with a "(p t) d"
    layout: each partition line is one contiguous 4 KB chunk -> 128
    descriptors per transfer (vs 1024x512B for the in-order layout),
    so issue cost is ~0.7 us and never blocks output issues.
  - That layout permutes patch order (position 128t+p holds patch
    8p+t).  The permutation is undone for free by scattering the stt
    (silu) write: bt columns are written in patch order via a strided
    output access pattern.  Everything downstream is identical to the
    natural layout.
  - Exp uses ACT accum_out for the row sums (no DVE tensor_reduce).
  - Per head g, emission order is: input-DMA(g+3) [tensor queue],
    softmax(g) [MM -> exp+accum -> normalize -> out-DMA], then
    transposes+tanh+stt(g+1), so head g's matmuls are never queued
    behind future heads' transposes in the PE FIFO.
  - Output tiles are written in 1 MB pairs (2 x 128 rows) -> 32 output
    DMAs, each 256 x 4KB descriptors.

Per-head math (all ACT work in the exp_and_others table set, so no
ACT table reloads): xT = PE-transpose(x) in PSUM fp32; tanh(x/2) via
ACT; bT = (tanh+1)*xT = 2*silu(x)^T via DVE stt (bf16, patch-ordered
scatter); scores = qbT_m^T @ kbT on PE (x4 factor folded into the exp
scale = scale/4); exp via ACT with accum row sums; normalize_recip on
GpSimd; DMA out.
"""

import numpy as np

B, H, P, D = 4, 16, 1024, 128
N_CORES = 8
G = (B * H) // N_CORES  # heads per core = 8
PT = P // 128  # p-tiles per head = 8

_cached = {}


def _build_module(mm_dtype_name="bfloat16"):
    import concourse.bass as bass
    import concourse.tile as tile
    from concourse import bacc, mybir
    from concourse.masks import make_identity

    f32 = mybir.dt.float32
    mm_dt = getattr(mybir.dt, mm_dtype_name)
    AF = mybir.ActivationFunctionType

    nc = bacc.Bacc("TRN2", target_bir_lowering=False, debug=False)
    q_d = nc.dram_tensor("q", [G, P, D], f32, kind="ExternalInput")
    k_d = nc.dram_tensor("k", [G, P, D], f32, kind="ExternalInput")
    scale_d = nc.dram_tensor("scale", [1], f32, kind="ExternalInput")
    out_d = nc.dram_tensor("out", [G, P, P], f32, kind="ExternalOutput")

    with tile.TileContext(nc) as tc:
        with (
            tc.tile_pool(name="consts", bufs=1) as consts,
            tc.tile_pool(name="nat", bufs=8) as natp,
            tc.tile_pool(name="th", bufs=4) as thp,
            tc.tile_pool(name="bt", bufs=4) as btp,
            tc.tile_pool(name="exp", bufs=6) as expp,
            tc.tile_pool(name="outs", bufs=5) as outp,
            tc.tile_pool(name="stats", bufs=8) as statp,
            tc.tile_pool(name="ps_t", bufs=2, space="PSUM") as ps_tp,
            tc.tile_pool(name="ps_s", bufs=2, space="PSUM") as ps_sp,
        ):
            identity = consts.tile([128, 128], f32)
            make_identity(nc, identity)
            scale_sb = consts.tile([128, 1], f32)
            nc.gpsimd.dma_start(out=scale_sb, in_=scale_d[:].to_broadcast([128, 1]))
            # bT = 2*silu => scores are 4x; fold the 1/4 into the exp scale
            scale_adj = consts.tile([128, 1], f32)
            nc.vector.tensor_scalar_mul(scale_adj, scale_sb, 0.25)

            nats = {}

            def dma_in(g):
                """Input DMA for head g. Partition p holds rows [8p, 8p+8)
                as one contiguous 4 KB chunk -> 128 descriptors, so the
                HWDGE issue is ~0.65 us and does not block output issues
                on the shared sync queue. (Both alternate queues measured
                WORSE: gpsimd/SWDGE descriptor-gen + completion work on Q7
                delayed NormalizeRecip by ~4.6 us; the scalar/Act HWDGE
                ring cost ~7 us by stalling ACT compute dispatch.)"""
                pair = {}
                for nm, src in (("k", k_d), ("q", q_d)):
                    nat = natp.tile([128, PT, 128], f32, tag="nat", name=f"nat_{nm}{g}")
                    nc.sync.dma_start(
                        out=nat, in_=src[g].rearrange("(p t) d -> p t d", p=128)
                    )
                    pair[nm] = nat
                nats[g] = pair

            def prep_transpose(g, nm):
                """PE-transpose one input tensor of head g into PSUM.
                The "(p t)" layout leaves columns in permuted order
                (position 128t+p <- patch 8p+t)."""
                nat = nats[g][nm]
                ps_t = ps_tp.tile([128, P], f32, tag="ps_t", name=f"psT_{nm}{g}")
                for t in range(PT):
                    nc.tensor.transpose(
                        ps_t[:, bass.ts(t, 128)], nat[:, t, :], identity
                    )
                return ps_t

            def prep_silu(g, nm, ps_t):
                """tanh + stt: bt = (tanh(xT/2)+1)*xT = 2*silu(x)^T, bf16.
                bt is written contiguous in PATCH order; the layout
                permutation is absorbed in STRIDED READS of th/ps_t (the
                DVE fallback path reads strided fp32/PSUM operands at 1
                elem/cycle, while strided writes are ~4x slower)."""
                # tanh(x/2) in the exp_and_others table set
                th = thp.tile([128, P], mm_dt, tag="th", name=f"th_{nm}{g}")
                nc.scalar.activation(out=th, in_=ps_t, func=AF.Tanh, scale=0.5)
                bt = btp.tile([128, P], mm_dt, tag=f"bt_{nm}", name=f"bt_{nm}{g}")
                nc.vector.scalar_tensor_tensor(
                    out=bt[:, :].rearrange("r (p t) -> r p t", t=PT),
                    in0=th[:, :].rearrange("r (t p) -> r p t", t=PT),
                    scalar=1.0,
                    in1=ps_t[:, :].rearrange("r (t p) -> r p t", t=PT),
                    op0=mybir.AluOpType.add,
                    op1=mybir.AluOpType.mult,
                )
                return bt

            def prep_compute(g):
                bts = {}
                for nm in ("k", "q"):
                    bts[nm] = prep_silu(g, nm, prep_transpose(g, nm))
                del nats[g]
                return bts["q"], bts["k"]

            DI = 3  # input-DMA prefetch depth (heads)
            dma_in(0)
            ready = prep_compute(0)
            for g in range(1, DI):
                dma_in(g)

            for g in range(G):
                qbT, kbT = ready

                for mm in range(PT // 2):
                    out_t = outp.tile([128, 2, P], f32, tag="out", name=f"out_{g}_{mm}")
                    for a in range(2):
                        m = 2 * mm + a
                        ps_s = ps_sp.tile([128, P], f32, tag="ps_s", name=f"psS_{g}_{m}")
                        for h in range(2):
                            nc.tensor.matmul(
                                ps_s[:, bass.ts(h, 512)],
                                qbT[:, bass.ts(m, 128)],
                                kbT[:, bass.ts(h, 512)],
                                start=True,
                                stop=True,
                            )
                        exp_t = expp.tile([128, P], f32, tag="exp", name=f"exp_{g}_{m}")
                        sum_t = statp.tile([128, 1], f32, tag="sum", name=f"sum_{g}_{m}")
                        if m < 3:
                            # ACT accumulates row sums for free (+185ns
                            # ACTIVATION_READ_ACCUMULATOR per tile)
                            nc.scalar.activation(
                                out=exp_t,
                                in_=ps_s,
                                func=AF.Exp,
                                scale=scale_adj,
                                accum_out=sum_t,
                            )
                        else:
                            # ACT is the pacing engine; offload half the row
                            # sums to DVE (which has slack) to balance.
                            nc.scalar.activation(
                                out=exp_t, in_=ps_s, func=AF.Exp, scale=scale_adj
                            )
                            nc.vector.tensor_reduce(
                                out=sum_t,
                                in_=exp_t,
                                axis=mybir.AxisListType.X,
                                op=mybir.AluOpType.add,
                            )
                        nc.gpsimd.normalize_recip(out_t[:, a, :], exp_t, sum_t)
                    nc.sync.dma_start(
                        out=out_d[g, bass.ts(mm, 256), :].rearrange(
                            "(a p) c -> p a c", a=2
                        ),
                        in_=out_t,
                    )
                    # Interleave next head's input DMA + prep in small
                    # chunks across this head, so the tanh/stt land MID-head
                    # in the ACT/DVE FIFOs, each PE transpose block (~1.3us)
                    # is absorbed by the 2-deep score-PSUM backlog, and
                    # bt(g+1) is ready before head g's exps finish.
                    if g + 1 < G:
                        if mm == 0:
                            if g + DI < G:
                                dma_in(g + DI)
                            _psk = prep_transpose(g + 1, "k")
                        elif mm == 1:
                            _btk = prep_silu(g + 1, "k", _psk)
                            _psq = prep_transpose(g + 1, "q")
                        elif mm == 2:
                            _btq = prep_silu(g + 1, "q", _psq)
                            del nats[g + 1]
                            ready = (_btq, _btk)

    nc.compile()
    return nc


def _get_nc():
    if "nc" not in _cached:
        _cached["nc"] = _build_module()
    return _cached["nc"]


def kernel(q, k, scale, _trace=False):
    from concourse.bass_utils import run_bass_kernel_spmd

    nc = _get_nc()
    qf = np.ascontiguousarray(q.reshape(B * H, P, D), dtype=np.float32)
    kf = np.ascontiguousarray(k.reshape(B * H, P, D), dtype=np.float32)
    sc = np.ascontiguousarray(scale.reshape(1), dtype=np.float32)
    in_maps = [
        {"q": qf[i * G : (i + 1) * G], "k": kf[i * G : (i + 1) * G], "scale": sc}
        for i in range(N_CORES)
    ]
    res = run_bass_kernel_spmd(
        nc, in_maps, core_ids=list(range(N_CORES)), trace=_trace
    )
    out = np.concatenate([res.results[i]["out"] for i in range(N_CORES)], axis=0)
    if _trace:
        kernel.last_result = res
    return out.reshape(B, H, P, P)
